# revision 1
# baseline (speedup 1.0000x reference)
"""EnhancedEntityNBFNet (B=2, K=33, N=50000, E=800000, R=200, D=64, L=3)
on 8 Trainium2 NeuronCores — frontier-compacted Bass kernel.

Algorithm
---------
The reference's boundary condition is an indicator: per batch query only the
head entity h0 carries a (query-relation) vector; every other node row of x
starts exactly 0.  With layer_b == ln_b == 0 (how the inputs are generated),
an all-zero node row stays exactly 0 through every layer:
    LN(0) = 0,  relu(0) = 0,  0 + 0 = 0   (exact in fp32)
so after layer l the node state x_l is supported only on the l-hop
out-frontier of h0 (~17 nodes after layer 1, ~300 after layer 2), and the
final scores need hidden state at only the K=33 tail candidates.

The host does *integer* work only: BFS frontier sets, edge subsets, and 0/1
structure (scatter/gather/one-hot) matrices.  ALL floating-point math runs
on-device as dense matmuls over the compacted, padded active sets:
  - relation gather  rel[type_e]      ->  Tm^T @ rel        (one-hot matmul)
  - source gather    x[src_e]         ->  G^T @ x
  - distmult msg     gather * gather  ->  elementwise mul (VectorE)
  - segment_sum      sum_e->n msg     ->  S^T @ msg
  - boundary add     1[n=h0] q        ->  h0ind^T @ qv (PSUM accumulate)
  - update           cat @ W + b      ->  PE transpose + matmul
  - LayerNorm        bn_stats/bn_aggr + rsqrt, g/b broadcast via DMA step-0
  - relu/residual    ScalarE/VectorE
  - final MLP        relu([x3,q] @ W1 + b1) @ W2 + b2

Sharding: core c in 0..7 handles batch b = c//4 and one quarter of the K
tail candidates (the sharding_hint's batch split, plus a target split since
B=2 < 8).  Each core runs the same SPMD program on its own compacted inputs;
the host concatenates the per-core score rows back to the full [B, K] output.

Sizes are measured from the actual input graph at call time and padded to
multiples of 32 (>= 32), so the same program covers any frontier the random
graph produces; block loops generalize past 128.
"""
import sys

import numpy as np

for _p in ("/opt/trn_rl_repo", "/root/.axon_site/_ro/trn_rl_repo"):
    if _p not in sys.path:
        sys.path.insert(0, _p)

from contextlib import ExitStack

import concourse.bacc as bacc
import concourse.tile as tile
from concourse import mybir
from concourse.bass_utils import run_bass_kernel_spmd
from concourse.masks import make_identity

F32 = mybir.dt.float32
P = 128
D = 64          # hidden dim
RP = 256        # relation table rows, padded (R=200 -> 256)
L = 3           # layers
EPS = 1e-5
N_CORES = 8


def _pad32(n: int) -> int:
    return max(32, ((int(n) + 31) // 32) * 32)


def _blk(n):
    return [(o, min(P, n - o)) for o in range(0, n, P)]


# --------------------------------------------------------------------------
# host-side integer prep: frontier sets + structure matrices
# --------------------------------------------------------------------------

def _prep_host(rel, batch, edge_index, edge_type):
    src = np.asarray(edge_index[0], np.int64)
    dst = np.asarray(edge_index[1], np.int64)
    B = rel.shape[0]
    K = batch.shape[1]

    per_batch = []
    for b in range(B):
        h0 = int(batch[b, 0, 0])
        r0 = int(batch[b, 0, 2])
        e1 = np.nonzero(src == h0)[0]
        V1 = np.unique(np.concatenate([[h0], dst[e1]]))
        A2 = np.union1d(V1, dst[np.isin(src, V1)])
        per_batch.append(dict(h0=h0, r0=r0, e1=e1, V1=V1, A2=A2))

    cpb = N_CORES // B  # cores per batch
    chunks = np.array_split(np.arange(K), cpb)
    cores = []
    for c in range(N_CORES):
        b = c // cpb
        pb = per_batch[b]
        chunk = chunks[c % cpb]
        Tc = batch[b, chunk, 1].astype(np.int64)
        e3 = np.nonzero(np.isin(dst, Tc) & np.isin(src, pb["A2"]))[0]
        V2 = np.unique(np.concatenate([Tc, src[e3]]))
        e2 = np.nonzero(np.isin(dst, V2) & np.isin(src, pb["V1"]))[0]
        cores.append(dict(b=b, Tc=Tc, e2=e2, e3=e3, V2=V2, chunk=chunk))

    dims = dict(
        M1=_pad32(max(len(pb["V1"]) for pb in per_batch)),
        Q1=_pad32(max(len(pb["e1"]) for pb in per_batch)),
        M2=_pad32(max(len(ci["V2"]) for ci in cores)),
        Q2=_pad32(max(len(ci["e2"]) for ci in cores)),
        Q3=_pad32(max(len(ci["e3"]) for ci in cores)),
        KC=_pad32(max(len(ci["Tc"]) for ci in cores)),
    )
    return per_batch, cores, dims, (src, dst, np.asarray(edge_type, np.int64))


def _core_in_map(inputs, rel, pb, ci, dims, graph):
    src, dst, et = graph
    M1, Q1, M2, Q2, Q3, KC = (dims[k] for k in ("M1", "Q1", "M2", "Q2", "Q3", "KC"))

    h0, r0, e1, V1 = pb["h0"], pb["r0"], pb["e1"], pb["V1"]
    Tc, e2, e3, V2 = ci["Tc"], ci["e2"], ci["e3"], ci["V2"]
    b = ci["b"]

    pos1 = {n: i for i, n in enumerate(V1)}
    pos2 = {n: i for i, n in enumerate(V2)}
    q1, q2, q3, kc = len(e1), len(e2), len(e3), len(Tc)

    rel_pad = np.zeros((RP, D), np.float32)
    rel_pad[: rel.shape[1]] = rel[b]
    r0hot = np.zeros((RP, 1), np.float32)
    r0hot[r0, 0] = 1.0

    S1T = np.zeros((Q1, M1), np.float32)
    Tm1T = np.zeros((RP, Q1), np.float32)
    if q1:
        S1T[np.arange(q1), [pos1[n] for n in dst[e1]]] = 1.0
        Tm1T[et[e1], np.arange(q1)] = 1.0
    h0i1 = np.zeros((1, M1), np.float32)
    h0i1[0, pos1[h0]] = 1.0

    G2T = np.zeros((M1, Q2), np.float32)
    S2T = np.zeros((Q2, M2), np.float32)
    Tm2T = np.zeros((RP, Q2), np.float32)
    if q2:
        G2T[[pos1[n] for n in src[e2]], np.arange(q2)] = 1.0
        S2T[np.arange(q2), [pos2[n] for n in dst[e2]]] = 1.0
        Tm2T[et[e2], np.arange(q2)] = 1.0
    G12T = np.zeros((M1, M2), np.float32)
    for n in V2:
        if n in pos1:
            G12T[pos1[n], pos2[n]] = 1.0
    h0i2 = np.zeros((1, M2), np.float32)
    if h0 in pos2:
        h0i2[0, pos2[h0]] = 1.0

    G3T = np.zeros((M2, Q3), np.float32)
    S3T = np.zeros((Q3, KC), np.float32)
    Tm3T = np.zeros((RP, Q3), np.float32)
    if q3:
        G3T[[pos2[n] for n in src[e3]], np.arange(q3)] = 1.0
        S3T[:q3, :kc] = (dst[e3][:, None] == Tc[None, :]).astype(np.float32)
        Tm3T[et[e3], np.arange(q3)] = 1.0
    G23T = np.zeros((M2, KC), np.float32)
    G23T[[pos2[n] for n in Tc], np.arange(kc)] = 1.0
    h0i3 = np.zeros((1, KC), np.float32)
    h0i3[0, :kc] = (Tc == h0).astype(np.float32)

    return dict(
        rel=np.ascontiguousarray(rel_pad),
        r0hot=r0hot,
        layer_w=np.ascontiguousarray(np.asarray(inputs["layer_w"], np.float32)),
        layer_b=np.ascontiguousarray(np.asarray(inputs["layer_b"], np.float32)),
        ln_g=np.ascontiguousarray(np.asarray(inputs["ln_g"], np.float32)),
        ln_b=np.ascontiguousarray(np.asarray(inputs["ln_b"], np.float32)),
        mlp_w1=np.ascontiguousarray(np.asarray(inputs["mlp_w1"], np.float32)),
        mlp_b1=np.asarray(inputs["mlp_b1"], np.float32).reshape(D, 1).copy(),
        mlp_w2=np.ascontiguousarray(np.asarray(inputs["mlp_w2"], np.float32)),
        mlp_b2=np.asarray(inputs["mlp_b2"], np.float32).reshape(1, 1).copy(),
        S1T=S1T, Tm1T=Tm1T, h0ind1=h0i1,
        G2T=G2T, S2T=S2T, Tm2T=Tm2T, G12T=G12T, h0ind2=h0i2,
        G3T=G3T, S3T=S3T, Tm3T=Tm3T, G23T=G23T, h0ind3=h0i3,
    )


# --------------------------------------------------------------------------
# device program (identical SPMD program on all 8 cores)
# --------------------------------------------------------------------------

def build_nc(dims):
    M1, Q1, M2, Q2, Q3, KC = (dims[k] for k in ("M1", "Q1", "M2", "Q2", "Q3", "KC"))
    nc = bacc.Bacc()

    def din(name, shape):
        return nc.declare_dram_parameter(name, list(shape), F32, isOutput=False)

    rel = din("rel", (RP, D))
    r0hot = din("r0hot", (RP, 1))
    lw = din("layer_w", (L, 2 * D, D))
    lb = din("layer_b", (L, D))
    lng = din("ln_g", (L, D))
    lnb = din("ln_b", (L, D))
    w1 = din("mlp_w1", (2 * D, D))
    b1 = din("mlp_b1", (D, 1))
    w2 = din("mlp_w2", (D, 1))
    b2 = din("mlp_b2", (1, 1))
    s1t = din("S1T", (Q1, M1))
    tm1 = din("Tm1T", (RP, Q1))
    h01 = din("h0ind1", (1, M1))
    g2t = din("G2T", (M1, Q2))
    s2t = din("S2T", (Q2, M2))
    tm2 = din("Tm2T", (RP, Q2))
    g12 = din("G12T", (M1, M2))
    h02 = din("h0ind2", (1, M2))
    g3t = din("G3T", (M2, Q3))
    s3t = din("S3T", (Q3, KC))
    tm3 = din("Tm3T", (RP, Q3))
    g23 = din("G23T", (M2, KC))
    h03 = din("h0ind3", (1, KC))
    score = nc.declare_dram_parameter("score", [1, KC], F32, isOutput=True)

    with ExitStack() as ctx:
        tc = ctx.enter_context(tile.TileContext(nc))
        const = ctx.enter_context(tc.tile_pool(name="const", bufs=1))
        tmp = ctx.enter_context(tc.tile_pool(name="tmp", bufs=2))
        pps = ctx.enter_context(tc.tile_pool(name="ps", bufs=2, space="PSUM"))

        ident = const.tile([P, P], F32, tag="ident")
        make_identity(nc, ident[:])
        ones_row = const.tile([1, P], F32, tag="ones_row")
        nc.vector.memset(ones_row[:], 1.0)
        eps_t = const.tile([P, 1], F32, tag="eps")
        nc.vector.memset(eps_t[:], EPS)

        def load(dram, rows, cols, tag):
            out = []
            for i, (o, sz) in enumerate(_blk(rows)):
                t = const.tile([P, cols], F32, tag=f"{tag}{i}")
                nc.sync.dma_start(out=t[:sz, :cols], in_=dram[o : o + sz, 0:cols])
                out.append((t, sz))
            return out

        rel_b = load(rel, RP, D, "rel")
        r0h_b = load(r0hot, RP, 1, "r0h")
        tm1_b = load(tm1, RP, Q1, "tm1")
        tm2_b = load(tm2, RP, Q2, "tm2")
        tm3_b = load(tm3, RP, Q3, "tm3")
        s1t_b = load(s1t, Q1, M1, "s1t")
        s2t_b = load(s2t, Q2, M2, "s2t")
        s3t_b = load(s3t, Q3, KC, "s3t")
        g2t_b = load(g2t, M1, Q2, "g2t")
        g12_b = load(g12, M1, M2, "g12")
        g3t_b = load(g3t, M2, Q3, "g3t")
        g23_b = load(g23, M2, KC, "g23")
        h01_sb = load(h01, 1, M1, "h01")[0][0]
        h02_sb = load(h02, 1, M2, "h02")[0][0]
        h03_sb = load(h03, 1, KC, "h03")[0][0]

        w_sb = [load(lw[l], 2 * D, D, f"w{l}")[0][0] for l in range(L)]
        lb_sb = [load(lb[l : l + 1], 1, D, f"lb{l}")[0][0] for l in range(L)]
        w1_sb = load(w1, 2 * D, D, "w1")[0][0]
        b1_sb = load(b1, D, 1, "b1")[0][0]
        w2_sb = load(w2, D, 1, "w2")[0][0]
        b2_sb = load(b2, 1, 1, "b2")[0][0]

        gbc, bbc = [], []
        for l in range(L):
            g = const.tile([P, D], F32, tag=f"gbc{l}")
            nc.sync.dma_start(out=g[:, :D], in_=lng[l].partition_broadcast(P))
            gbc.append(g)
            bb = const.tile([P, D], F32, tag=f"bbc{l}")
            nc.sync.dma_start(out=bb[:, :D], in_=lnb[l].partition_broadcast(P))
            bbc.append(bb)

        # ---- query vector: qv = r0hot^T rel, plus transposed/broadcast copies
        qv_ps = pps.tile([1, D], F32, tag="ps_c")
        for i, ((rt, rs), (ht, _)) in enumerate(zip(rel_b, r0h_b)):
            nc.tensor.matmul(out=qv_ps[:1, :D], lhsT=ht[:rs, :1], rhs=rt[:rs, :D],
                             start=(i == 0), stop=(i == len(rel_b) - 1))
        qv = const.tile([1, D], F32, tag="qv")
        nc.vector.tensor_copy(out=qv[:1, :D], in_=qv_ps[:1, :D])

        qvT_ps = pps.tile([D, 1], F32, tag="ps_c")
        for i, ((rt, rs), (ht, _)) in enumerate(zip(rel_b, r0h_b)):
            nc.tensor.matmul(out=qvT_ps[:D, :1], lhsT=rt[:rs, :D], rhs=ht[:rs, :1],
                             start=(i == 0), stop=(i == len(rel_b) - 1))
        qvT = const.tile([D, 1], F32, tag="qvT")
        nc.vector.tensor_copy(out=qvT[:D, :1], in_=qvT_ps[:D, :1])

        qbc_ps = pps.tile([P, D], F32, tag="ps_a")
        nc.tensor.matmul(out=qbc_ps[:P, :D], lhsT=ones_row[:1, :P], rhs=qv[:1, :D],
                         start=True, stop=True)
        qbc = const.tile([P, D], F32, tag="qbc")
        nc.vector.tensor_copy(out=qbc[:, :D], in_=qbc_ps[:, :D])

        def ln_relu_res(u, ms, l, xprev, xout):
            """xout[:ms] = relu(LN(u)*g_l+b_l) + xprev[:ms]   (u: SBUF [P,D])"""
            stats = tmp.tile([P, 6], F32, tag="stats")
            mv = tmp.tile([P, 2], F32, tag="mv")
            nc.vector.bn_stats(out=stats[:ms, :], in_=u[:ms, :D])
            nc.vector.bn_aggr(out=mv[:ms, :], in_=stats[:ms, :])
            mean = mv[:ms, 0:1]
            var = mv[:ms, 1:2]
            nc.scalar.activation(out=var, in_=var,
                                 func=mybir.ActivationFunctionType.Sqrt,
                                 bias=eps_t[:ms], scale=1.0)
            nc.vector.reciprocal(out=var, in_=var)
            nc.vector.tensor_scalar(out=u[:ms, :D], in0=u[:ms, :D],
                                    scalar1=mean, scalar2=var,
                                    op0=mybir.AluOpType.subtract,
                                    op1=mybir.AluOpType.mult)
            nc.vector.tensor_mul(out=u[:ms, :D], in0=u[:ms, :D], in1=gbc[l][:ms, :D])
            nc.vector.tensor_add(out=u[:ms, :D], in0=u[:ms, :D], in1=bbc[l][:ms, :D])
            nc.scalar.activation(out=u[:ms, :D], in_=u[:ms, :D],
                                 func=mybir.ActivationFunctionType.Relu)
            nc.vector.tensor_add(out=xout[:ms, :D], in0=u[:ms, :D], in1=xprev[:ms, :D])

        def dense_update(xcat, ms, l, xprev, xout):
            """xout = relu(LN(xcat @ W_l + b_l)) + xprev for one node block."""
            xT_ps = pps.tile([P, P], F32, tag="ps_b")
            nc.tensor.transpose(out=xT_ps[: 2 * D, :ms], in_=xcat[:ms, : 2 * D],
                                identity=ident[:ms, :ms])
            xT = tmp.tile([P, P], F32, tag="xT")
            nc.vector.tensor_copy(out=xT[: 2 * D, :ms], in_=xT_ps[: 2 * D, :ms])
            upd_ps = pps.tile([P, D], F32, tag="ps_a")
            nc.tensor.matmul(out=upd_ps[:ms, :D], lhsT=xT[: 2 * D, :ms],
                             rhs=w_sb[l][: 2 * D, :D], start=True, stop=False)
            nc.tensor.matmul(out=upd_ps[:ms, :D], lhsT=ones_row[:1, :ms],
                             rhs=lb_sb[l][:1, :D], start=False, stop=True)
            u = tmp.tile([P, D], F32, tag="u")
            nc.vector.tensor_copy(out=u[:ms, :D], in_=upd_ps[:ms, :D])
            ln_relu_res(u, ms, l, xprev, xout)

        def msgs(tm_b, g_b, x_blocks, Q, tag):
            """edge messages: (G^T x_prev) * (Tm^T rel); layer 1 (x_blocks
            None) uses the query broadcast for the source factor."""
            out = []
            for j, (qo, qs) in enumerate(_blk(Q)):
                tr_ps = pps.tile([P, D], F32, tag="ps_a")
                for i, (rt, rs) in enumerate(rel_b):
                    nc.tensor.matmul(out=tr_ps[:qs, :D],
                                     lhsT=tm_b[i][0][:rs, qo : qo + qs],
                                     rhs=rt[:rs, :D],
                                     start=(i == 0), stop=(i == len(rel_b) - 1))
                m = const.tile([P, D], F32, tag=f"{tag}_{j}")
                if x_blocks is None:
                    nc.vector.tensor_mul(out=m[:qs, :D], in0=tr_ps[:qs, :D],
                                         in1=qbc[:qs, :D])
                else:
                    gx_ps = pps.tile([P, D], F32, tag="ps_b")
                    for i, (xt, ms_) in enumerate(x_blocks):
                        nc.tensor.matmul(out=gx_ps[:qs, :D],
                                         lhsT=g_b[i][0][:ms_, qo : qo + qs],
                                         rhs=xt[:ms_, :D],
                                         start=(i == 0), stop=(i == len(x_blocks) - 1))
                    gx = tmp.tile([P, D], F32, tag="gx")
                    nc.vector.tensor_copy(out=gx[:qs, :D], in_=gx_ps[:qs, :D])
                    nc.vector.tensor_mul(out=m[:qs, :D], in0=tr_ps[:qs, :D],
                                         in1=gx[:qs, :D])
                out.append((m, qs))
            return out

        def aggregate(s_b, msg_blocks, h0_sb, mo, ms):
            """PSUM = S^T msg + h0ind^T qv (segment sum + boundary)."""
            agg_ps = pps.tile([P, D], F32, tag="ps_a")
            for j, (mt, qs) in enumerate(msg_blocks):
                nc.tensor.matmul(out=agg_ps[:ms, :D],
                                 lhsT=s_b[j][0][:qs, mo : mo + ms], rhs=mt[:qs, :D],
                                 start=(j == 0), stop=False)
            nc.tensor.matmul(out=agg_ps[:ms, :D], lhsT=h0_sb[:1, mo : mo + ms],
                             rhs=qv[:1, :D], start=False, stop=True)
            return agg_ps

        def gather_nodes(g_b, x_blocks, mo, ms, tag):
            """SBUF = G^T x_prev for one node block of the new node list."""
            ps = pps.tile([P, D], F32, tag="ps_b")
            for i, (xt, ms_) in enumerate(x_blocks):
                nc.tensor.matmul(out=ps[:ms, :D], lhsT=g_b[i][0][:ms_, mo : mo + ms],
                                 rhs=xt[:ms_, :D],
                                 start=(i == 0), stop=(i == len(x_blocks) - 1))
            t = const.tile([P, D], F32, tag=tag)
            nc.vector.tensor_copy(out=t[:ms, :D], in_=ps[:ms, :D])
            return t

        # ---- layer 1 ----
        msg1 = msgs(tm1_b, None, None, Q1, "msg1")
        x1 = []
        for mi, (mo, ms) in enumerate(_blk(M1)):
            agg_ps = aggregate(s1t_b, msg1, h01_sb, mo, ms)
            x0_ps = pps.tile([P, D], F32, tag="ps_b")
            nc.tensor.matmul(out=x0_ps[:ms, :D], lhsT=h01_sb[:1, mo : mo + ms],
                             rhs=qv[:1, :D], start=True, stop=True)
            x0 = const.tile([P, D], F32, tag=f"x0_{mi}")
            nc.vector.tensor_copy(out=x0[:ms, :D], in_=x0_ps[:ms, :D])
            xcat = tmp.tile([P, 2 * D], F32, tag="xcat")
            nc.vector.tensor_copy(out=xcat[:ms, :D], in_=agg_ps[:ms, :D])
            nc.vector.tensor_copy(out=xcat[:ms, D : 2 * D], in_=x0[:ms, :D])
            xo = const.tile([P, D], F32, tag=f"x1_{mi}")
            dense_update(xcat, ms, 0, x0, xo)
            x1.append((xo, ms))

        # ---- layer 2 ----
        msg2 = msgs(tm2_b, g2t_b, x1, Q2, "msg2")
        x2 = []
        for mi, (mo, ms) in enumerate(_blk(M2)):
            agg_ps = aggregate(s2t_b, msg2, h02_sb, mo, ms)
            xp = gather_nodes(g12_b, x1, mo, ms, f"x1v2_{mi}")
            xcat = tmp.tile([P, 2 * D], F32, tag="xcat")
            nc.vector.tensor_copy(out=xcat[:ms, :D], in_=agg_ps[:ms, :D])
            nc.vector.tensor_copy(out=xcat[:ms, D : 2 * D], in_=xp[:ms, :D])
            xo = const.tile([P, D], F32, tag=f"x2_{mi}")
            dense_update(xcat, ms, 1, xp, xo)
            x2.append((xo, ms))

        # ---- layer 3: only the target slots ----
        msg3 = msgs(tm3_b, g3t_b, x2, Q3, "msg3")
        x3 = []
        for mi, (mo, ms) in enumerate(_blk(KC)):
            agg_ps = aggregate(s3t_b, msg3, h03_sb, mo, ms)
            xp = gather_nodes(g23_b, x2, mo, ms, f"x2v3_{mi}")
            xcat = tmp.tile([P, 2 * D], F32, tag="xcat")
            nc.vector.tensor_copy(out=xcat[:ms, :D], in_=agg_ps[:ms, :D])
            nc.vector.tensor_copy(out=xcat[:ms, D : 2 * D], in_=xp[:ms, :D])
            xo = const.tile([P, D], F32, tag=f"x3_{mi}")
            dense_update(xcat, ms, 2, xp, xo)
            x3.append((xo, ms))

        # ---- final MLP: score = relu([x3, q] @ w1 + b1) @ w2 + b2 ----
        for (x3t, ms), (mo, _) in zip(x3, _blk(KC)):
            x3T_ps = pps.tile([P, P], F32, tag="ps_b")
            nc.tensor.transpose(out=x3T_ps[:D, :ms], in_=x3t[:ms, :D],
                                identity=ident[:ms, :ms])
            featT = tmp.tile([P, P], F32, tag="featT")
            nc.vector.tensor_copy(out=featT[:D, :ms], in_=x3T_ps[:D, :ms])
            nc.vector.tensor_copy(out=featT[D : 2 * D, :ms],
                                  in_=qvT[:D, :1].to_broadcast([D, ms]))
            h_ps = pps.tile([D, P], F32, tag="ps_a")
            nc.tensor.matmul(out=h_ps[:D, :ms], lhsT=w1_sb[: 2 * D, :D],
                             rhs=featT[: 2 * D, :ms], start=True, stop=True)
            h = tmp.tile([D, P], F32, tag="h")
            nc.vector.tensor_scalar(out=h[:D, :ms], in0=h_ps[:D, :ms],
                                    scalar1=b1_sb[:D, :1], scalar2=None,
                                    op0=mybir.AluOpType.add)
            nc.scalar.activation(out=h[:D, :ms], in_=h[:D, :ms],
                                 func=mybir.ActivationFunctionType.Relu)
            sc_ps = pps.tile([1, P], F32, tag="ps_c")
            nc.tensor.matmul(out=sc_ps[:1, :ms], lhsT=w2_sb[:D, :1],
                             rhs=h[:D, :ms], start=True, stop=True)
            sc = tmp.tile([1, P], F32, tag="sc")
            nc.vector.tensor_scalar(out=sc[:1, :ms], in0=sc_ps[:1, :ms],
                                    scalar1=b2_sb[:1, :1], scalar2=None,
                                    op0=mybir.AluOpType.add)
            nc.sync.dma_start(out=score[0:1, mo : mo + ms], in_=sc[:1, :ms])

    nc.finalize()
    return nc


# --------------------------------------------------------------------------
# numpy fallback (only taken if the zero-bias structural assumption fails,
# which the input spec's fills rule out; kept for correctness insurance)
# --------------------------------------------------------------------------

def _dense_numpy(inputs):
    rel = np.asarray(inputs["relation_representations"], np.float32)
    lw = np.asarray(inputs["layer_w"], np.float32)
    lbv = np.asarray(inputs["layer_b"], np.float32)
    lng = np.asarray(inputs["ln_g"], np.float32)
    lnb = np.asarray(inputs["ln_b"], np.float32)
    batch = np.asarray(inputs["batch"])
    ei = np.asarray(inputs["edge_index"])
    et = np.asarray(inputs["edge_type"])
    N = int(inputs["num_nodes"])
    B, R, D_ = rel.shape
    h0 = batch[:, 0, 0].astype(np.int64)
    r0 = batch[:, 0, 2].astype(np.int64)
    t = batch[:, :, 1].astype(np.int64)
    query = rel[np.arange(B), r0]
    boundary = np.zeros((B, N, D_), np.float32)
    boundary[np.arange(B), h0] += query
    src, dst = ei[0], ei[1]
    x = boundary.copy()
    for l in range(lw.shape[0]):
        msg = x[:, src] * rel[:, et]
        agg = np.zeros_like(x)
        np.add.at(agg, (slice(None), dst), msg)
        agg += boundary
        u = np.concatenate([agg, x], -1) @ lw[l] + lbv[l]
        mu = u.mean(-1, keepdims=True)
        var = ((u - mu) ** 2).mean(-1, keepdims=True)
        u = (u - mu) / np.sqrt(var + EPS) * lng[l] + lnb[l]
        x = np.maximum(u, 0) + x
    feat_t = np.take_along_axis(
        np.concatenate([x, np.broadcast_to(query[:, None, :], x.shape)], -1),
        t[..., None], axis=1)
    w1 = np.asarray(inputs["mlp_w1"], np.float32)
    b1 = np.asarray(inputs["mlp_b1"], np.float32)
    w2 = np.asarray(inputs["mlp_w2"], np.float32)
    b2 = np.asarray(inputs["mlp_b2"], np.float32)
    return ((np.maximum(feat_t @ w1 + b1, 0) @ w2 + b2)[..., 0]).astype(np.float32)


# --------------------------------------------------------------------------
# public entry
# --------------------------------------------------------------------------

def kernel(**inputs) -> np.ndarray:
    rel = np.asarray(inputs["relation_representations"], np.float32)
    batch = np.asarray(inputs["batch"])
    B, K = batch.shape[0], batch.shape[1]

    # zero-row invariance needs layer_b == ln_b == 0 (true per input spec)
    if not (np.all(np.asarray(inputs["layer_b"]) == 0)
            and np.all(np.asarray(inputs["ln_b"]) == 0)) or N_CORES % B:
        return _dense_numpy(inputs)

    per_batch, cores, dims, graph = _prep_host(
        rel, batch, np.asarray(inputs["edge_index"]),
        np.asarray(inputs["edge_type"]))
    in_maps = [_core_in_map(inputs, rel, per_batch[ci["b"]], ci, dims, graph)
               for ci in cores]
    nc = build_nc(dims)
    res = run_bass_kernel_spmd(nc, in_maps, list(range(N_CORES)))
    out = np.zeros((B, K), np.float32)
    for c, ci in enumerate(cores):
        out[ci["b"], ci["chunk"]] = res.results[c]["score"][0, : len(ci["Tc"])]
    return out


# revision 2
# speedup vs baseline: 2.1115x; 2.1115x over previous
"""EnhancedEntityNBFNet (B=2, K=33, N=50000, E=800000, R=200, D=64, L=3)
on 8 Trainium2 NeuronCores — frontier-compacted Bass kernel.

Algorithm
---------
The reference's boundary condition is an indicator: per batch query only the
head entity h0 carries a (query-relation) vector; every other node row of x
starts exactly 0.  With layer_b == ln_b == 0 (how the inputs are generated),
an all-zero node row stays exactly 0 through every layer:
    LN(0) = 0,  relu(0) = 0,  0 + 0 = 0   (exact in fp32)
so after layer l the node state x_l is supported only on the l-hop
out-frontier of h0 (~20 nodes after layer 1, ~300 after layer 2), and the
final scores need hidden state at only the K=33 tail candidates.

The host does *integer/structure* work only: frontier sets, edge subsets, 0/1
scatter/gather/one-hot matrices, and an integer type-count matrix.  ALL
floating-point math runs on-device as dense matmuls over the compacted,
padded active sets:
  - layer-1 aggregate   rel^T @ C1        (C1[r,n] = #edges h0->n of type r;
                        an all-ones rel row carries the boundary indicator)
  - relation gather     Tm^T @ rel        (one-hot matmul)
  - source gather       G^T @ x
  - distmult message    gather * gather   (VectorE)
  - segment_sum         S^T @ msg  (+ boundary via qv outer product, PSUM)
  - update              cat @ W (computed via the transposed concat built
                        directly in PSUM -> no PE transposes in the layers)
  - LayerNorm           bn_stats/bn_aggr + Sqrt + reciprocal
  - relu/residual       VectorE (max, add)
  - final MLP           relu(x3 @ W1top + q @ W1bot) @ W2, one PE transpose

All per-core inputs are packed into two DRAM blobs (2 input DMAs/core).
Zero biases / unit LN gains (per the input spec) let the builder skip the
corresponding ops; the flags re-enable them if the inputs say otherwise.

Sharding: core c in 0..7 handles batch b = c//4 and one quarter of the K
tail candidates (the sharding_hint's batch split, plus a target split since
B=2 < 8).  All 8 cores run one SPMD program on their own compacted inputs;
the host concatenates the per-core score rows into the full [B, K] output.

Sizes are measured from the actual input graph at call time (the program is
compiled per call), padded to multiples of 32.  If a frontier exceeds the
fast path's 128 limit, a general block-tiled variant of the same program
runs instead; if the zero-bias invariance doesn't hold, a dense numpy
fallback guarantees correctness.
"""
import sys

import numpy as np

for _p in ("/opt/trn_rl_repo", "/root/.axon_site/_ro/trn_rl_repo"):
    if _p not in sys.path:
        sys.path.insert(0, _p)

from contextlib import ExitStack

import concourse.bacc as bacc
import concourse.tile as tile
from concourse import mybir
from concourse.bass_utils import run_bass_kernel_spmd
from concourse.masks import make_identity

F32 = mybir.dt.float32
P = 128
D = 64          # hidden dim
RP = 256        # relation table rows, padded (R=200 -> 256)
L = 3           # layers
EPS = 1e-5
N_CORES = 8


def _pad32(n: int) -> int:
    return max(32, ((int(n) + 31) // 32) * 32)


def _blk(n):
    return [(o, min(P, n - o)) for o in range(0, n, P)]


# --------------------------------------------------------------------------
# host-side integer prep: frontier sets
# --------------------------------------------------------------------------

def _prep_host(rel, batch, edge_index, edge_type):
    src = np.asarray(edge_index[0], np.int64)
    dst = np.asarray(edge_index[1], np.int64)
    B = rel.shape[0]
    K = batch.shape[1]

    per_batch = []
    for b in range(B):
        h0 = int(batch[b, 0, 0])
        r0 = int(batch[b, 0, 2])
        e1 = np.nonzero(src == h0)[0]
        V1 = np.unique(np.concatenate([[h0], dst[e1]]))
        A2 = np.union1d(V1, dst[np.isin(src, V1)])
        per_batch.append(dict(h0=h0, r0=r0, e1=e1, V1=V1, A2=A2))

    cpb = N_CORES // B  # cores per batch
    chunks = np.array_split(np.arange(K), cpb)
    cores = []
    for c in range(N_CORES):
        b = c // cpb
        pb = per_batch[b]
        chunk = chunks[c % cpb]
        Tc = batch[b, chunk, 1].astype(np.int64)
        e3 = np.nonzero(np.isin(dst, Tc) & np.isin(src, pb["A2"]))[0]
        V2 = np.unique(np.concatenate([Tc, src[e3]]))
        e2 = np.nonzero(np.isin(dst, V2) & np.isin(src, pb["V1"]))[0]
        cores.append(dict(b=b, Tc=Tc, e2=e2, e3=e3, V2=V2, chunk=chunk))

    dims = dict(
        M1=_pad32(max(len(pb["V1"]) for pb in per_batch)),
        Q1=_pad32(max(len(pb["e1"]) for pb in per_batch)),
        M2=_pad32(max(len(ci["V2"]) for ci in cores)),
        Q2=_pad32(max(len(ci["e2"]) for ci in cores)),
        Q3=_pad32(max(len(ci["e3"]) for ci in cores)),
        KC=_pad32(max(len(ci["Tc"]) for ci in cores)),
    )
    return per_batch, cores, dims, (src, dst, np.asarray(edge_type, np.int64))


def _core_structs(inputs, rel, pb, ci, dims, graph, build_l1):
    """One core's 0/1 structure matrices + padded rel table (all float32)."""
    src, dst, et = graph
    M1, Q1, M2, Q2, Q3, KC = (dims[k] for k in ("M1", "Q1", "M2", "Q2", "Q3", "KC"))

    h0, r0, e1, V1 = pb["h0"], pb["r0"], pb["e1"], pb["V1"]
    Tc, e2, e3, V2 = ci["Tc"], ci["e2"], ci["e3"], ci["V2"]
    b = ci["b"]
    R = rel.shape[1]

    pos1 = {n: i for i, n in enumerate(V1)}
    pos2 = {n: i for i, n in enumerate(V2)}
    q1, q2, q3, kc = len(e1), len(e2), len(e3), len(Tc)

    rel_pad = np.zeros((RP, D), np.float32)
    rel_pad[:R] = rel[b]
    r0hot = np.zeros((RP, 1), np.float32)
    r0hot[r0, 0] = 1.0

    h0i1 = np.zeros((1, M1), np.float32)
    h0i1[0, pos1[h0]] = 1.0
    out = dict(rel=rel_pad, r0hot=r0hot, h0ind1=h0i1)

    # layer-1 type-count matrix C1[r, n] = #edges h0 -> V1[n] of type r
    c1 = np.zeros((RP, M1), np.float32)
    if q1:
        np.add.at(c1, (et[e1], [pos1[n] for n in dst[e1]]), 1.0)
    out["C1"] = c1

    if build_l1:  # explicit edge matrices for the block-tiled fallback
        S1T = np.zeros((Q1, M1), np.float32)
        Tm1T = np.zeros((RP, Q1), np.float32)
        if q1:
            S1T[np.arange(q1), [pos1[n] for n in dst[e1]]] = 1.0
            Tm1T[et[e1], np.arange(q1)] = 1.0
        out["S1T"], out["Tm1T"] = S1T, Tm1T

    G2T = np.zeros((M1, Q2), np.float32)
    S2T = np.zeros((Q2, M2), np.float32)
    Tm2T = np.zeros((RP, Q2), np.float32)
    if q2:
        G2T[[pos1[n] for n in src[e2]], np.arange(q2)] = 1.0
        S2T[np.arange(q2), [pos2[n] for n in dst[e2]]] = 1.0
        Tm2T[et[e2], np.arange(q2)] = 1.0
    G12T = np.zeros((M1, M2), np.float32)
    for n in V2:
        if n in pos1:
            G12T[pos1[n], pos2[n]] = 1.0
    h0i2 = np.zeros((1, M2), np.float32)
    if h0 in pos2:
        h0i2[0, pos2[h0]] = 1.0

    G3T = np.zeros((M2, Q3), np.float32)
    S3T = np.zeros((Q3, KC), np.float32)
    Tm3T = np.zeros((RP, Q3), np.float32)
    if q3:
        G3T[[pos2[n] for n in src[e3]], np.arange(q3)] = 1.0
        S3T[:q3, :kc] = (dst[e3][:, None] == Tc[None, :]).astype(np.float32)
        Tm3T[et[e3], np.arange(q3)] = 1.0
    G23T = np.zeros((M2, KC), np.float32)
    G23T[[pos2[n] for n in Tc], np.arange(kc)] = 1.0
    h0i3 = np.zeros((1, KC), np.float32)
    h0i3[0, :kc] = (Tc == h0).astype(np.float32)

    out.update(G2T=G2T, S2T=S2T, Tm2T=Tm2T, G12T=G12T, h0ind2=h0i2,
               G3T=G3T, S3T=S3T, Tm3T=Tm3T, G23T=G23T, h0ind3=h0i3)
    return out


def _flags(inputs):
    return dict(
        ln_affine=not (np.all(np.asarray(inputs["ln_g"]) == 1)
                       and np.all(np.asarray(inputs["ln_b"]) == 0)),
        layer_bias=bool(np.any(np.asarray(inputs["layer_b"]) != 0)),
        mlp_bias=bool(np.any(np.asarray(inputs["mlp_b1"]) != 0)
                      or np.any(np.asarray(inputs["mlp_b2"]) != 0)),
    )


# --------------------------------------------------------------------------
# fast path (all active dims <= 128): packed blobs, 2 input DMAs per core
# --------------------------------------------------------------------------

def _seg(cols):
    out, off = {}, 0
    for name, w in cols:
        out[name] = (off, w)
        off += w
    return out, off


def _prep_blobs(inputs, rel, per_batch, cores, dims, graph):
    M1, M2, Q2, Q3, KC = (dims[k] for k in ("M1", "M2", "Q2", "Q3", "KC"))
    R = rel.shape[1]
    segA, CA = _seg([("rel", D), ("r0hot", 1), ("c1", M1), ("tm2", Q2), ("tm3", Q3)])
    segB, CB = _seg([("s2t", M2), ("s3t", KC), ("g2t", Q2),
                     ("g12", M2), ("g3t", Q3), ("g23", KC),
                     ("w0", D), ("w1", D), ("w2", D),
                     ("mw1", D), ("mw1b", D), ("mw2", 1), ("mb1", 1)])
    segC, CC = _seg([("h01", M1), ("h02", M2), ("h03", KC),
                     ("lb0", D), ("lb1", D), ("lb2", D), ("mb2", 1)])
    lw = np.asarray(inputs["layer_w"], np.float32)
    lbv = np.asarray(inputs["layer_b"], np.float32)
    mw1 = np.asarray(inputs["mlp_w1"], np.float32)
    mw2 = np.asarray(inputs["mlp_w2"], np.float32)
    mb1 = np.asarray(inputs["mlp_b1"], np.float32).reshape(D, 1)
    mb2 = np.asarray(inputs["mlp_b2"], np.float32).reshape(1, 1)
    lngv = np.ascontiguousarray(np.asarray(inputs["ln_g"], np.float32))
    lnbv = np.ascontiguousarray(np.asarray(inputs["ln_b"], np.float32))

    in_maps = []
    for ci in cores:
        im = _core_structs(inputs, rel, per_batch[ci["b"]], ci, dims, graph,
                           build_l1=False)
        A = np.zeros((RP, CA), np.float32)

        def putA(name, arr):
            o, w = segA[name]
            A[: arr.shape[0], o : o + w] = arr

        rel_aug = im["rel"].copy()
        rel_aug[R, :] = 1.0            # spare all-ones row for the boundary
        c1 = im["C1"]
        c1[R, :] = im["h0ind1"][0]     # boundary rides the ones-row of rel
        putA("rel", rel_aug)
        putA("r0hot", im["r0hot"])
        putA("c1", c1)
        putA("tm2", im["Tm2T"])
        putA("tm3", im["Tm3T"])
        A = np.concatenate([A[:P], A[P:]], axis=1)  # interleave -> [128, 2*CA]

        Bb = np.zeros((P, CB), np.float32)

        def putB(name, arr):
            o, w = segB[name]
            Bb[: arr.shape[0], o : o + w] = arr

        putB("s2t", im["S2T"])
        putB("s3t", im["S3T"])
        putB("g2t", im["G2T"])
        putB("g12", im["G12T"])
        putB("g3t", im["G3T"])
        putB("g23", im["G23T"])
        for l in range(L):
            putB(f"w{l}", lw[l])
        putB("mw1", mw1)
        putB("mw1b", mw1[D:])
        putB("mw2", mw2)
        putB("mb1", mb1)

        Cc = np.zeros((1, CC), np.float32)

        def putC(name, arr):
            o, w = segC[name]
            Cc[:, o : o + w] = arr.reshape(1, -1)

        putC("h01", im["h0ind1"])
        putC("h02", im["h0ind2"])
        putC("h03", im["h0ind3"])
        for l in range(L):
            putC(f"lb{l}", lbv[l])
        putC("mb2", mb2)
        BC = np.concatenate(
            [Bb, np.concatenate([Cc, np.zeros((P - 1, CC), np.float32)], 0)], 1)
        in_maps.append(dict(blobA=np.ascontiguousarray(A),
                            blobB=np.ascontiguousarray(BC),
                            ln_g=lngv, ln_b=lnbv))
    return in_maps, (segA, CA), (segB, CB), (segC, CC)


def _build_nc_fast(dims, layA, layB, layC, flags):
    M1, M2, Q2, Q3, KC = (dims[k] for k in ("M1", "M2", "Q2", "Q3", "KC"))
    segA, CA = layA
    segB, CB = layB
    segC, CC = layC
    nc = bacc.Bacc()
    blobA = nc.declare_dram_parameter("blobA", [P, 2 * CA], F32, isOutput=False)
    blobB = nc.declare_dram_parameter("blobB", [P, CB + CC], F32, isOutput=False)
    lng = nc.declare_dram_parameter("ln_g", [L, D], F32, isOutput=False)
    lnb = nc.declare_dram_parameter("ln_b", [L, D], F32, isOutput=False)
    score = nc.declare_dram_parameter("score", [1, KC], F32, isOutput=True)

    with ExitStack() as ctx:
        tc = ctx.enter_context(tile.TileContext(nc))
        const = ctx.enter_context(tc.tile_pool(name="const", bufs=1))
        tmp = ctx.enter_context(tc.tile_pool(name="tmp", bufs=2))
        pps = ctx.enter_context(tc.tile_pool(name="ps", bufs=2, space="PSUM"))

        ident = const.tile([P, P], F32, tag="ident")
        make_identity(nc, ident[:])
        ones_row = const.tile([1, P], F32, tag="ones_row")
        nc.vector.memset(ones_row[:], 1.0)
        eps_t = const.tile([P, 1], F32, tag="eps")
        nc.vector.memset(eps_t[:], EPS)

        tA = const.tile([P, 2 * CA], F32, tag="tA")
        nc.sync.dma_start(out=tA[:, :], in_=blobA[0:P, :])
        tB = const.tile([P, CB + CC], F32, tag="tB")
        nc.gpsimd.dma_start(out=tB[:, :], in_=blobB[:, :])
        if flags["ln_affine"]:
            tG = const.tile([P, L * D], F32, tag="tG")
            nc.sync.dma_start(
                out=tG[:, :],
                in_=lng.rearrange("a b -> (a b)").partition_broadcast(P))
            tBB = const.tile([P, L * D], F32, tag="tBB")
            nc.sync.dma_start(
                out=tBB[:, :],
                in_=lnb.rearrange("a b -> (a b)").partition_broadcast(P))

        def A0(name):
            o, w = segA[name]
            return tA[:, o : o + w]

        def A1(name):
            o, w = segA[name]
            return tA[:, CA + o : CA + o + w]

        def Bs(name, rows=P):
            o, w = segB[name]
            return tB[:rows, o : o + w]

        def Cs(name):
            o, w = segC[name]
            return tB[:1, CB + o : CB + o + w]

        # ---- query vector: qv [1,D] and its transpose qvT [D,1]
        qv_ps = pps.tile([1, D], F32, tag="ps_c")
        nc.tensor.matmul(out=qv_ps[:1, :D], lhsT=A0("r0hot"), rhs=A0("rel"),
                         start=True, stop=False)
        nc.tensor.matmul(out=qv_ps[:1, :D], lhsT=A1("r0hot"), rhs=A1("rel"),
                         start=False, stop=True)
        qv = const.tile([1, D], F32, tag="qv")
        nc.vector.tensor_copy(out=qv[:1, :D], in_=qv_ps[:1, :D])
        qvT_ps = pps.tile([D, 1], F32, tag="ps_c")
        nc.tensor.matmul(out=qvT_ps[:D, :1], lhsT=A0("rel"), rhs=A0("r0hot"),
                         start=True, stop=False)
        nc.tensor.matmul(out=qvT_ps[:D, :1], lhsT=A1("rel"), rhs=A1("r0hot"),
                         start=False, stop=True)
        qvT = const.tile([D, 1], F32, tag="qvT")
        nc.vector.tensor_copy(out=qvT[:D, :1], in_=qvT_ps[:D, :1])

        def trel(tm_name, Q, tag):
            """TmRel [Q, D] = Tm^T @ rel in SBUF (per-edge relation rows)."""
            ps = pps.tile([P, D], F32, tag="ps_a")
            nc.tensor.matmul(out=ps[:Q, :D], lhsT=A0(tm_name), rhs=A0("rel"),
                             start=True, stop=False)
            nc.tensor.matmul(out=ps[:Q, :D], lhsT=A1(tm_name), rhs=A1("rel"),
                             start=False, stop=True)
            t = const.tile([P, D], F32, tag=tag)
            nc.vector.tensor_copy(out=t[:Q, :D], in_=ps[:Q, :D])
            return t

        def layer_norm_relu(u_ps, m, l, res_ps, xout):
            """xout[:m] = relu(LN(u_ps) [*g+b]) + res_ps  (u_ps/res_ps PSUM)"""
            stats = tmp.tile([P, 6], F32, tag="stats")
            mv = tmp.tile([P, 2], F32, tag="mv")
            nc.vector.bn_stats(out=stats[:m, :], in_=u_ps[:m, :D])
            nc.vector.bn_aggr(out=mv[:m, :], in_=stats[:m, :])
            mean = mv[:m, 0:1]
            var = mv[:m, 1:2]
            nc.scalar.activation(out=var, in_=var,
                                 func=mybir.ActivationFunctionType.Sqrt,
                                 bias=eps_t[:m], scale=1.0)
            nc.vector.reciprocal(out=var, in_=var)
            u = tmp.tile([P, D], F32, tag="u")
            nc.vector.tensor_scalar(out=u[:m, :D], in0=u_ps[:m, :D],
                                    scalar1=mean, scalar2=var,
                                    op0=mybir.AluOpType.subtract,
                                    op1=mybir.AluOpType.mult)
            if flags["ln_affine"]:
                nc.vector.tensor_mul(out=u[:m, :D], in0=u[:m, :D],
                                     in1=tG[:m, l * D : (l + 1) * D])
                nc.vector.tensor_add(out=u[:m, :D], in0=u[:m, :D],
                                     in1=tBB[:m, l * D : (l + 1) * D])
            nc.vector.tensor_scalar_max(out=u[:m, :D], in0=u[:m, :D],
                                        scalar1=0.0)
            nc.vector.tensor_add(out=xout[:m, :D], in0=u[:m, :D],
                                 in1=res_ps[:m, :D])

        def dense(xcatT, m, l):
            """PSUM [m, D] = Xcat @ W_l (+ b_l)."""
            ps = pps.tile([P, D], F32, tag="ps_a")
            nc.tensor.matmul(out=ps[:m, :D], lhsT=xcatT[: 2 * D, :m],
                             rhs=Bs(f"w{l}"), start=True,
                             stop=not flags["layer_bias"])
            if flags["layer_bias"]:
                nc.tensor.matmul(out=ps[:m, :D], lhsT=ones_row[:1, :m],
                                 rhs=Cs(f"lb{l}"), start=False, stop=True)
            return ps

        # =========== layer 1 ===========
        # catT rows 0:64  <- (rel^T @ C1) * qvT   (= agg1^T incl. boundary)
        # catT rows 64:128 <- qvT (x) h0ind1      (= x0^T)
        cat_ps = pps.tile([P, P], F32, tag="ps_b")
        nc.tensor.matmul(out=cat_ps[:D, :M1], lhsT=A0("rel"), rhs=A0("c1"),
                         start=True, stop=False)
        nc.tensor.matmul(out=cat_ps[:D, :M1], lhsT=A1("rel"), rhs=A1("c1"),
                         start=False, stop=True)
        nc.tensor.matmul(out=cat_ps[D : 2 * D, :M1], lhsT=qv[:1, :D],
                         rhs=Cs("h01"), start=True, stop=True)
        catT = tmp.tile([P, P], F32, tag="catT")
        nc.vector.tensor_mul(out=catT[:D, :M1], in0=cat_ps[:D, :M1],
                             in1=qvT[:D, :1].to_broadcast([D, M1]))
        nc.vector.tensor_copy(out=catT[D : 2 * D, :M1],
                              in_=cat_ps[D : 2 * D, :M1])
        u_ps = dense(catT, M1, 0)
        x0_ps = pps.tile([P, D], F32, tag="ps_b")
        nc.tensor.matmul(out=x0_ps[:M1, :D], lhsT=Cs("h01"), rhs=qv[:1, :D],
                         start=True, stop=True)
        x1 = const.tile([P, D], F32, tag="x1")
        layer_norm_relu(u_ps, M1, 0, x0_ps, x1)

        def mp_layer(tm_name, g_name, s_name, gp_name, h0_name, x_prev, Mp,
                     Q, Mn, l, xtag):
            """message-passing layer: returns x_l tile [Mn, D]."""
            trl = trel(tm_name, Q, f"trel{l}")
            gx_ps = pps.tile([P, D], F32, tag="ps_b")
            nc.tensor.matmul(out=gx_ps[:Q, :D], lhsT=Bs(g_name, Mp),
                             rhs=x_prev[:Mp, :D], start=True, stop=True)
            msg = const.tile([P, D], F32, tag=f"msg{l}")
            nc.vector.tensor_mul(out=msg[:Q, :D], in0=gx_ps[:Q, :D],
                                 in1=trl[:Q, :D])
            cat_ps = pps.tile([P, P], F32, tag="ps_b")
            nc.tensor.matmul(out=cat_ps[:D, :Mn], lhsT=msg[:Q, :D],
                             rhs=Bs(s_name, Q), start=True, stop=False)
            nc.tensor.matmul(out=cat_ps[:D, :Mn], lhsT=qv[:1, :D],
                             rhs=Cs(h0_name), start=False, stop=True)
            nc.tensor.matmul(out=cat_ps[D : 2 * D, :Mn], lhsT=x_prev[:Mp, :D],
                             rhs=Bs(gp_name, Mp), start=True, stop=True)
            catT = tmp.tile([P, P], F32, tag="catT")
            nc.vector.tensor_copy(out=catT[: 2 * D, :Mn],
                                  in_=cat_ps[: 2 * D, :Mn])
            u_ps = dense(catT, Mn, l)
            xp_ps = pps.tile([P, D], F32, tag="ps_b")
            nc.tensor.matmul(out=xp_ps[:Mn, :D], lhsT=Bs(gp_name, Mp),
                             rhs=x_prev[:Mp, :D], start=True, stop=True)
            xo = const.tile([P, D], F32, tag=xtag)
            layer_norm_relu(u_ps, Mn, l, xp_ps, xo)
            return xo

        x2 = mp_layer("tm2", "g2t", "s2t", "g12", "h02", x1, M1, Q2, M2, 1, "x2")
        x3 = mp_layer("tm3", "g3t", "s3t", "g23", "h03", x2, M2, Q3, KC, 2, "x3")

        # =========== final MLP: relu(x3 @ W1top + q @ W1bot) @ W2 (+ b) =====
        qwb_ps = pps.tile([1, D], F32, tag="ps_c")
        nc.tensor.matmul(out=qwb_ps[:1, :D], lhsT=qvT[:D, :1],
                         rhs=Bs("mw1b", D), start=True, stop=True)
        qwb = const.tile([1, D], F32, tag="qwb")
        nc.vector.tensor_copy(out=qwb[:1, :D], in_=qwb_ps[:1, :D])
        x3T_ps = pps.tile([P, P], F32, tag="ps_b")
        nc.tensor.transpose(out=x3T_ps[:D, :KC], in_=x3[:KC, :D],
                            identity=ident[:KC, :KC])
        x3T = tmp.tile([D, P], F32, tag="x3T")
        nc.vector.tensor_copy(out=x3T[:D, :KC], in_=x3T_ps[:D, :KC])
        h_ps = pps.tile([D, P], F32, tag="ps_a")
        nc.tensor.matmul(out=h_ps[:D, :KC], lhsT=Bs("mw1")[:D, :D],
                         rhs=x3T[:D, :KC], start=True, stop=False)
        nc.tensor.matmul(out=h_ps[:D, :KC], lhsT=qwb[:1, :D],
                         rhs=ones_row[:1, :KC], start=False, stop=True)
        h = tmp.tile([D, P], F32, tag="h")
        if flags["mlp_bias"]:
            nc.vector.tensor_scalar(out=h[:D, :KC], in0=h_ps[:D, :KC],
                                    scalar1=Bs("mb1", D), scalar2=None,
                                    op0=mybir.AluOpType.add)
            nc.vector.tensor_scalar_max(out=h[:D, :KC], in0=h[:D, :KC],
                                        scalar1=0.0)
        else:
            nc.vector.tensor_scalar_max(out=h[:D, :KC], in0=h_ps[:D, :KC],
                                        scalar1=0.0)
        sc_ps = pps.tile([1, P], F32, tag="ps_c")
        nc.tensor.matmul(out=sc_ps[:1, :KC], lhsT=Bs("mw2", D),
                         rhs=h[:D, :KC], start=True, stop=True)
        sc = tmp.tile([1, P], F32, tag="sc")
        if flags["mlp_bias"]:
            nc.vector.tensor_scalar(out=sc[:1, :KC], in0=sc_ps[:1, :KC],
                                    scalar1=Cs("mb2"), scalar2=None,
                                    op0=mybir.AluOpType.add)
        else:
            nc.vector.tensor_copy(out=sc[:1, :KC], in_=sc_ps[:1, :KC])
        nc.sync.dma_start(out=score[0:1, :KC], in_=sc[:1, :KC])

    nc.finalize()
    return nc


# --------------------------------------------------------------------------
# general fallback: block-tiled variant (any frontier size)
# --------------------------------------------------------------------------

def _core_in_map_general(inputs, rel, pb, ci, dims, graph):
    im = _core_structs(inputs, rel, pb, ci, dims, graph, build_l1=True)
    return dict(
        rel=np.ascontiguousarray(im["rel"]),
        r0hot=im["r0hot"],
        layer_w=np.ascontiguousarray(np.asarray(inputs["layer_w"], np.float32)),
        layer_b=np.ascontiguousarray(np.asarray(inputs["layer_b"], np.float32)),
        ln_g=np.ascontiguousarray(np.asarray(inputs["ln_g"], np.float32)),
        ln_b=np.ascontiguousarray(np.asarray(inputs["ln_b"], np.float32)),
        mlp_w1=np.ascontiguousarray(np.asarray(inputs["mlp_w1"], np.float32)),
        mlp_b1=np.asarray(inputs["mlp_b1"], np.float32).reshape(D, 1).copy(),
        mlp_w2=np.ascontiguousarray(np.asarray(inputs["mlp_w2"], np.float32)),
        mlp_b2=np.asarray(inputs["mlp_b2"], np.float32).reshape(1, 1).copy(),
        S1T=im["S1T"], Tm1T=im["Tm1T"], h0ind1=im["h0ind1"],
        G2T=im["G2T"], S2T=im["S2T"], Tm2T=im["Tm2T"], G12T=im["G12T"],
        h0ind2=im["h0ind2"], G3T=im["G3T"], S3T=im["S3T"], Tm3T=im["Tm3T"],
        G23T=im["G23T"], h0ind3=im["h0ind3"],
    )


def _build_nc_general(dims):
    M1, Q1, M2, Q2, Q3, KC = (dims[k] for k in ("M1", "Q1", "M2", "Q2", "Q3", "KC"))
    nc = bacc.Bacc()

    def din(name, shape):
        return nc.declare_dram_parameter(name, list(shape), F32, isOutput=False)

    rel = din("rel", (RP, D))
    r0hot = din("r0hot", (RP, 1))
    lw = din("layer_w", (L, 2 * D, D))
    lb = din("layer_b", (L, D))
    lng = din("ln_g", (L, D))
    lnb = din("ln_b", (L, D))
    w1 = din("mlp_w1", (2 * D, D))
    b1 = din("mlp_b1", (D, 1))
    w2 = din("mlp_w2", (D, 1))
    b2 = din("mlp_b2", (1, 1))
    s1t = din("S1T", (Q1, M1))
    tm1 = din("Tm1T", (RP, Q1))
    h01 = din("h0ind1", (1, M1))
    g2t = din("G2T", (M1, Q2))
    s2t = din("S2T", (Q2, M2))
    tm2 = din("Tm2T", (RP, Q2))
    g12 = din("G12T", (M1, M2))
    h02 = din("h0ind2", (1, M2))
    g3t = din("G3T", (M2, Q3))
    s3t = din("S3T", (Q3, KC))
    tm3 = din("Tm3T", (RP, Q3))
    g23 = din("G23T", (M2, KC))
    h03 = din("h0ind3", (1, KC))
    score = nc.declare_dram_parameter("score", [1, KC], F32, isOutput=True)

    with ExitStack() as ctx:
        tc = ctx.enter_context(tile.TileContext(nc))
        const = ctx.enter_context(tc.tile_pool(name="const", bufs=1))
        tmp = ctx.enter_context(tc.tile_pool(name="tmp", bufs=2))
        pps = ctx.enter_context(tc.tile_pool(name="ps", bufs=2, space="PSUM"))

        ident = const.tile([P, P], F32, tag="ident")
        make_identity(nc, ident[:])
        ones_row = const.tile([1, P], F32, tag="ones_row")
        nc.vector.memset(ones_row[:], 1.0)
        eps_t = const.tile([P, 1], F32, tag="eps")
        nc.vector.memset(eps_t[:], EPS)

        def load(dram, rows, cols, tag):
            out = []
            for i, (o, sz) in enumerate(_blk(rows)):
                t = const.tile([P, cols], F32, tag=f"{tag}{i}")
                nc.sync.dma_start(out=t[:sz, :cols], in_=dram[o : o + sz, 0:cols])
                out.append((t, sz))
            return out

        rel_b = load(rel, RP, D, "rel")
        r0h_b = load(r0hot, RP, 1, "r0h")
        tm1_b = load(tm1, RP, Q1, "tm1")
        tm2_b = load(tm2, RP, Q2, "tm2")
        tm3_b = load(tm3, RP, Q3, "tm3")
        s1t_b = load(s1t, Q1, M1, "s1t")
        s2t_b = load(s2t, Q2, M2, "s2t")
        s3t_b = load(s3t, Q3, KC, "s3t")
        g2t_b = load(g2t, M1, Q2, "g2t")
        g12_b = load(g12, M1, M2, "g12")
        g3t_b = load(g3t, M2, Q3, "g3t")
        g23_b = load(g23, M2, KC, "g23")
        h01_sb = load(h01, 1, M1, "h01")[0][0]
        h02_sb = load(h02, 1, M2, "h02")[0][0]
        h03_sb = load(h03, 1, KC, "h03")[0][0]

        w_sb = [load(lw[l], 2 * D, D, f"w{l}")[0][0] for l in range(L)]
        lb_sb = [load(lb[l : l + 1], 1, D, f"lb{l}")[0][0] for l in range(L)]
        w1_sb = load(w1, 2 * D, D, "w1")[0][0]
        b1_sb = load(b1, D, 1, "b1")[0][0]
        w2_sb = load(w2, D, 1, "w2")[0][0]
        b2_sb = load(b2, 1, 1, "b2")[0][0]

        gbc, bbc = [], []
        for l in range(L):
            g = const.tile([P, D], F32, tag=f"gbc{l}")
            nc.sync.dma_start(out=g[:, :D], in_=lng[l].partition_broadcast(P))
            gbc.append(g)
            bb = const.tile([P, D], F32, tag=f"bbc{l}")
            nc.sync.dma_start(out=bb[:, :D], in_=lnb[l].partition_broadcast(P))
            bbc.append(bb)

        qv_ps = pps.tile([1, D], F32, tag="ps_c")
        for i, ((rt, rs), (ht, _)) in enumerate(zip(rel_b, r0h_b)):
            nc.tensor.matmul(out=qv_ps[:1, :D], lhsT=ht[:rs, :1], rhs=rt[:rs, :D],
                             start=(i == 0), stop=(i == len(rel_b) - 1))
        qv = const.tile([1, D], F32, tag="qv")
        nc.vector.tensor_copy(out=qv[:1, :D], in_=qv_ps[:1, :D])

        qvT_ps = pps.tile([D, 1], F32, tag="ps_c")
        for i, ((rt, rs), (ht, _)) in enumerate(zip(rel_b, r0h_b)):
            nc.tensor.matmul(out=qvT_ps[:D, :1], lhsT=rt[:rs, :D], rhs=ht[:rs, :1],
                             start=(i == 0), stop=(i == len(rel_b) - 1))
        qvT = const.tile([D, 1], F32, tag="qvT")
        nc.vector.tensor_copy(out=qvT[:D, :1], in_=qvT_ps[:D, :1])

        qbc_ps = pps.tile([P, D], F32, tag="ps_a")
        nc.tensor.matmul(out=qbc_ps[:P, :D], lhsT=ones_row[:1, :P], rhs=qv[:1, :D],
                         start=True, stop=True)
        qbc = const.tile([P, D], F32, tag="qbc")
        nc.vector.tensor_copy(out=qbc[:, :D], in_=qbc_ps[:, :D])

        def ln_relu_res(u, ms, l, xprev, xout):
            stats = tmp.tile([P, 6], F32, tag="stats")
            mv = tmp.tile([P, 2], F32, tag="mv")
            nc.vector.bn_stats(out=stats[:ms, :], in_=u[:ms, :D])
            nc.vector.bn_aggr(out=mv[:ms, :], in_=stats[:ms, :])
            mean = mv[:ms, 0:1]
            var = mv[:ms, 1:2]
            nc.scalar.activation(out=var, in_=var,
                                 func=mybir.ActivationFunctionType.Sqrt,
                                 bias=eps_t[:ms], scale=1.0)
            nc.vector.reciprocal(out=var, in_=var)
            nc.vector.tensor_scalar(out=u[:ms, :D], in0=u[:ms, :D],
                                    scalar1=mean, scalar2=var,
                                    op0=mybir.AluOpType.subtract,
                                    op1=mybir.AluOpType.mult)
            nc.vector.tensor_mul(out=u[:ms, :D], in0=u[:ms, :D], in1=gbc[l][:ms, :D])
            nc.vector.tensor_add(out=u[:ms, :D], in0=u[:ms, :D], in1=bbc[l][:ms, :D])
            nc.vector.tensor_scalar_max(out=u[:ms, :D], in0=u[:ms, :D], scalar1=0.0)
            nc.vector.tensor_add(out=xout[:ms, :D], in0=u[:ms, :D],
                                 in1=xprev[:ms, :D])

        def dense_update(xcat, ms, l, xprev, xout):
            xT_ps = pps.tile([P, P], F32, tag="ps_b")
            nc.tensor.transpose(out=xT_ps[: 2 * D, :ms], in_=xcat[:ms, : 2 * D],
                                identity=ident[:ms, :ms])
            xT = tmp.tile([P, P], F32, tag="xT")
            nc.vector.tensor_copy(out=xT[: 2 * D, :ms], in_=xT_ps[: 2 * D, :ms])
            upd_ps = pps.tile([P, D], F32, tag="ps_a")
            nc.tensor.matmul(out=upd_ps[:ms, :D], lhsT=xT[: 2 * D, :ms],
                             rhs=w_sb[l][: 2 * D, :D], start=True, stop=False)
            nc.tensor.matmul(out=upd_ps[:ms, :D], lhsT=ones_row[:1, :ms],
                             rhs=lb_sb[l][:1, :D], start=False, stop=True)
            u = tmp.tile([P, D], F32, tag="u")
            nc.vector.tensor_copy(out=u[:ms, :D], in_=upd_ps[:ms, :D])
            ln_relu_res(u, ms, l, xprev, xout)

        def msgs(tm_b, g_b, x_blocks, Q, tag):
            out = []
            for j, (qo, qs) in enumerate(_blk(Q)):
                tr_ps = pps.tile([P, D], F32, tag="ps_a")
                for i, (rt, rs) in enumerate(rel_b):
                    nc.tensor.matmul(out=tr_ps[:qs, :D],
                                     lhsT=tm_b[i][0][:rs, qo : qo + qs],
                                     rhs=rt[:rs, :D],
                                     start=(i == 0), stop=(i == len(rel_b) - 1))
                m = const.tile([P, D], F32, tag=f"{tag}_{j}")
                if x_blocks is None:
                    nc.vector.tensor_mul(out=m[:qs, :D], in0=tr_ps[:qs, :D],
                                         in1=qbc[:qs, :D])
                else:
                    gx_ps = pps.tile([P, D], F32, tag="ps_b")
                    for i, (xt, ms_) in enumerate(x_blocks):
                        nc.tensor.matmul(out=gx_ps[:qs, :D],
                                         lhsT=g_b[i][0][:ms_, qo : qo + qs],
                                         rhs=xt[:ms_, :D],
                                         start=(i == 0),
                                         stop=(i == len(x_blocks) - 1))
                    gx = tmp.tile([P, D], F32, tag="gx")
                    nc.vector.tensor_copy(out=gx[:qs, :D], in_=gx_ps[:qs, :D])
                    nc.vector.tensor_mul(out=m[:qs, :D], in0=tr_ps[:qs, :D],
                                         in1=gx[:qs, :D])
                out.append((m, qs))
            return out

        def aggregate(s_b, msg_blocks, h0_sb, mo, ms):
            agg_ps = pps.tile([P, D], F32, tag="ps_a")
            for j, (mt, qs) in enumerate(msg_blocks):
                nc.tensor.matmul(out=agg_ps[:ms, :D],
                                 lhsT=s_b[j][0][:qs, mo : mo + ms], rhs=mt[:qs, :D],
                                 start=(j == 0), stop=False)
            nc.tensor.matmul(out=agg_ps[:ms, :D], lhsT=h0_sb[:1, mo : mo + ms],
                             rhs=qv[:1, :D], start=False, stop=True)
            return agg_ps

        def gather_nodes(g_b, x_blocks, mo, ms, tag):
            ps = pps.tile([P, D], F32, tag="ps_b")
            for i, (xt, ms_) in enumerate(x_blocks):
                nc.tensor.matmul(out=ps[:ms, :D], lhsT=g_b[i][0][:ms_, mo : mo + ms],
                                 rhs=xt[:ms_, :D],
                                 start=(i == 0), stop=(i == len(x_blocks) - 1))
            t = const.tile([P, D], F32, tag=tag)
            nc.vector.tensor_copy(out=t[:ms, :D], in_=ps[:ms, :D])
            return t

        # layer 1
        msg1 = msgs(tm1_b, None, None, Q1, "msg1")
        x1 = []
        for mi, (mo, ms) in enumerate(_blk(M1)):
            agg_ps = aggregate(s1t_b, msg1, h01_sb, mo, ms)
            x0_ps = pps.tile([P, D], F32, tag="ps_b")
            nc.tensor.matmul(out=x0_ps[:ms, :D], lhsT=h01_sb[:1, mo : mo + ms],
                             rhs=qv[:1, :D], start=True, stop=True)
            x0 = const.tile([P, D], F32, tag=f"x0_{mi}")
            nc.vector.tensor_copy(out=x0[:ms, :D], in_=x0_ps[:ms, :D])
            xcat = tmp.tile([P, 2 * D], F32, tag="xcat")
            nc.vector.tensor_copy(out=xcat[:ms, :D], in_=agg_ps[:ms, :D])
            nc.vector.tensor_copy(out=xcat[:ms, D : 2 * D], in_=x0[:ms, :D])
            xo = const.tile([P, D], F32, tag=f"x1_{mi}")
            dense_update(xcat, ms, 0, x0, xo)
            x1.append((xo, ms))

        # layer 2
        msg2 = msgs(tm2_b, g2t_b, x1, Q2, "msg2")
        x2 = []
        for mi, (mo, ms) in enumerate(_blk(M2)):
            agg_ps = aggregate(s2t_b, msg2, h02_sb, mo, ms)
            xp = gather_nodes(g12_b, x1, mo, ms, f"x1v2_{mi}")
            xcat = tmp.tile([P, 2 * D], F32, tag="xcat")
            nc.vector.tensor_copy(out=xcat[:ms, :D], in_=agg_ps[:ms, :D])
            nc.vector.tensor_copy(out=xcat[:ms, D : 2 * D], in_=xp[:ms, :D])
            xo = const.tile([P, D], F32, tag=f"x2_{mi}")
            dense_update(xcat, ms, 1, xp, xo)
            x2.append((xo, ms))

        # layer 3 (target slots)
        msg3 = msgs(tm3_b, g3t_b, x2, Q3, "msg3")
        x3 = []
        for mi, (mo, ms) in enumerate(_blk(KC)):
            agg_ps = aggregate(s3t_b, msg3, h03_sb, mo, ms)
            xp = gather_nodes(g23_b, x2, mo, ms, f"x2v3_{mi}")
            xcat = tmp.tile([P, 2 * D], F32, tag="xcat")
            nc.vector.tensor_copy(out=xcat[:ms, :D], in_=agg_ps[:ms, :D])
            nc.vector.tensor_copy(out=xcat[:ms, D : 2 * D], in_=xp[:ms, :D])
            xo = const.tile([P, D], F32, tag=f"x3_{mi}")
            dense_update(xcat, ms, 2, xp, xo)
            x3.append((xo, ms))

        # final MLP
        for (x3t, ms), (mo, _) in zip(x3, _blk(KC)):
            x3T_ps = pps.tile([P, P], F32, tag="ps_b")
            nc.tensor.transpose(out=x3T_ps[:D, :ms], in_=x3t[:ms, :D],
                                identity=ident[:ms, :ms])
            featT = tmp.tile([P, P], F32, tag="featT")
            nc.vector.tensor_copy(out=featT[:D, :ms], in_=x3T_ps[:D, :ms])
            nc.vector.tensor_copy(out=featT[D : 2 * D, :ms],
                                  in_=qvT[:D, :1].to_broadcast([D, ms]))
            h_ps = pps.tile([D, P], F32, tag="ps_a")
            nc.tensor.matmul(out=h_ps[:D, :ms], lhsT=w1_sb[: 2 * D, :D],
                             rhs=featT[: 2 * D, :ms], start=True, stop=True)
            h = tmp.tile([D, P], F32, tag="h")
            nc.vector.tensor_scalar(out=h[:D, :ms], in0=h_ps[:D, :ms],
                                    scalar1=b1_sb[:D, :1], scalar2=None,
                                    op0=mybir.AluOpType.add)
            nc.vector.tensor_scalar_max(out=h[:D, :ms], in0=h[:D, :ms],
                                        scalar1=0.0)
            sc_ps = pps.tile([1, P], F32, tag="ps_c")
            nc.tensor.matmul(out=sc_ps[:1, :ms], lhsT=w2_sb[:D, :1],
                             rhs=h[:D, :ms], start=True, stop=True)
            sc = tmp.tile([1, P], F32, tag="sc")
            nc.vector.tensor_scalar(out=sc[:1, :ms], in0=sc_ps[:1, :ms],
                                    scalar1=b2_sb[:1, :1], scalar2=None,
                                    op0=mybir.AluOpType.add)
            nc.sync.dma_start(out=score[0:1, mo : mo + ms], in_=sc[:1, :ms])

    nc.finalize()
    return nc


# --------------------------------------------------------------------------
# numpy fallback (only if the zero-bias structural assumption fails, which
# the input spec's fills rule out; kept for correctness insurance)
# --------------------------------------------------------------------------

def _dense_numpy(inputs):
    rel = np.asarray(inputs["relation_representations"], np.float32)
    lw = np.asarray(inputs["layer_w"], np.float32)
    lbv = np.asarray(inputs["layer_b"], np.float32)
    lng = np.asarray(inputs["ln_g"], np.float32)
    lnb = np.asarray(inputs["ln_b"], np.float32)
    batch = np.asarray(inputs["batch"])
    ei = np.asarray(inputs["edge_index"])
    et = np.asarray(inputs["edge_type"])
    N = int(inputs["num_nodes"])
    B = rel.shape[0]
    h0 = batch[:, 0, 0].astype(np.int64)
    r0 = batch[:, 0, 2].astype(np.int64)
    t = batch[:, :, 1].astype(np.int64)
    query = rel[np.arange(B), r0]
    boundary = np.zeros((B, N, rel.shape[2]), np.float32)
    boundary[np.arange(B), h0] += query
    src, dst = ei[0], ei[1]
    x = boundary.copy()
    for l in range(lw.shape[0]):
        msg = x[:, src] * rel[:, et]
        agg = np.zeros_like(x)
        np.add.at(agg, (slice(None), dst), msg)
        agg += boundary
        u = np.concatenate([agg, x], -1) @ lw[l] + lbv[l]
        mu = u.mean(-1, keepdims=True)
        var = ((u - mu) ** 2).mean(-1, keepdims=True)
        u = (u - mu) / np.sqrt(var + EPS) * lng[l] + lnb[l]
        x = np.maximum(u, 0) + x
    feat_t = np.take_along_axis(
        np.concatenate([x, np.broadcast_to(query[:, None, :], x.shape)], -1),
        t[..., None], axis=1)
    w1 = np.asarray(inputs["mlp_w1"], np.float32)
    b1 = np.asarray(inputs["mlp_b1"], np.float32)
    w2 = np.asarray(inputs["mlp_w2"], np.float32)
    b2 = np.asarray(inputs["mlp_b2"], np.float32)
    return ((np.maximum(feat_t @ w1 + b1, 0) @ w2 + b2)[..., 0]).astype(np.float32)


# --------------------------------------------------------------------------
# public entry
# --------------------------------------------------------------------------

def kernel(**inputs) -> np.ndarray:
    rel = np.asarray(inputs["relation_representations"], np.float32)
    batch = np.asarray(inputs["batch"])
    B, K = batch.shape[0], batch.shape[1]
    R = rel.shape[1]

    # zero-row invariance needs layer_b == ln_b == 0 (true per the input spec)
    if (not (np.all(np.asarray(inputs["layer_b"]) == 0)
             and np.all(np.asarray(inputs["ln_b"]) == 0))
            or N_CORES % B or rel.shape[2] != D or R >= RP):
        return _dense_numpy(inputs)

    per_batch, cores, dims, graph = _prep_host(
        rel, batch, np.asarray(inputs["edge_index"]),
        np.asarray(inputs["edge_type"]))
    flags = _flags(inputs)

    fast = max(dims["M1"], dims["M2"], dims["Q2"], dims["Q3"], dims["KC"]) <= P
    if not fast and max(dims.values()) > 4096:
        return _dense_numpy(inputs)  # pathological hub graph: stay correct

    if fast:
        in_maps, layA, layB, layC = _prep_blobs(
            inputs, rel, per_batch, cores, dims, graph)
        nc = _build_nc_fast(dims, layA, layB, layC, flags)
    else:
        in_maps = [_core_in_map_general(inputs, rel, per_batch[ci["b"]], ci,
                                        dims, graph) for ci in cores]
        nc = _build_nc_general(dims)

    res = run_bass_kernel_spmd(nc, in_maps, list(range(N_CORES)))
    out = np.zeros((B, K), np.float32)
    for c, ci in enumerate(cores):
        out[ci["b"], ci["chunk"]] = res.results[c]["score"][0, : len(ci["Tc"])]
    return out
